# revision 12
# baseline (speedup 1.0000x reference)
"""Trainium2 Bass kernel for the contextual-bandit routing module.

Math (collapsed form of the reference network; biases kept general):
  ctx[b]      = concat(cemb[x[b,0]], cemb[x[b,1]])                 # [2D=128]
  P_a         = W2[a] @ W1[a]            c_a = W2[a]@b1[a] + b2[a] # [D,2D], [D]
  preds[b,a]  = P_a @ ctx[b] + c_a
  Q_a         = Wp @ P_a + Wc            d_a = Wp @ c_a + br1      # [H,2D], [H]
  z[b,a]      = relu(Q_a @ ctx[b] + d_a)
  rewards     = Wr2 . z[b,a]  (+br2, irrelevant for argmin)
  out r[b]    = preds[b, argmin_a rewards]
  out wemb[b] = wemb_table[y[b]]

Sharding: data-parallel over batch across 8 cores; weights replicated.

v2 fast path (requires d == 0, i.e. b1 == br1 == 0 as the problem spec
declares): rewards are computed in a TRANSPOSED layout — samples on
partitions, (arm, H) along the free dim — so the per-arm reward dot
w . relu(z) becomes a DVE free-axis reduce instead of per-(arm,chunk)
LDWEIGHTS-bound matmuls.  |w| is folded into the Q rows on the host and
the H dims are reordered so all w>0 rows come first (w is shared across
arms): reward = sum(relu(z'))[pos block] - sum(relu(z'))[neg block].
relu is then a pure PSUM evacuation (no bias), split across ACT and DVE.
Selected preds are read straight out of PSUM by copy_predicated (one
tile of skew so masks are ready), so preds are never staged in SBUF.

v1 (general-bias) path kept as fallback for nonzero d.
"""

import sys

sys.path.insert(0, "/opt/trn_rl_repo")

from contextlib import ExitStack

import numpy as np

import concourse.bass as bass
import concourse.bacc as bacc
import concourse.mybir as mybir
import concourse.tile as tile
from concourse.bass_utils import run_bass_kernel_spmd

F32 = mybir.dt.float32
I32 = mybir.dt.int32
U8 = mybir.dt.uint8
U32 = mybir.dt.uint32
F32R = mybir.dt.float32r
BF16 = mybir.dt.bfloat16
F16 = mybir.dt.float16
U16 = mybir.dt.uint16

NCORES = 8
B, A, D, H, V = 32768, 32, 64, 128, 50000
D2 = 2 * D  # 128
BC = B // NCORES  # 4096 samples per core
TILE = 512  # samples per tile
NT = BC // TILE  # 8 tiles
CH = TILE // 128  # 4 chunks of 128 samples
NPAIR = A // 2  # 16 arm pairs
NG = A // 4  # 8 groups of 4 arms (rewards path)

# relu-evacuation split: of the 8 groups per chunk, this many go to ACT,
# the rest to DVE.
ACT_RELU = 8

_CACHE = {}


def _build_v2(p, batched_gather=False):
    """Transposed-rewards program. `p` = number of w>0 rows (H reordered
    pos-first on host). Requires d == 0."""
    nc = bacc.Bacc(
        "TRN2", target_bir_lowering=False, debug=False, num_devices=NCORES
    )

    cemb = nc.dram_tensor("cemb", [V, D], F32, kind="ExternalInput").ap()
    wembt = nc.dram_tensor("wembt", [V, D], F32, kind="ExternalInput").ap()
    xidx0_d = nc.dram_tensor("xidx0", [128, NT * CH], I32, kind="ExternalInput").ap()
    xidx1_d = nc.dram_tensor("xidx1", [128, NT * CH], I32, kind="ExternalInput").ap()
    yidx_d = nc.dram_tensor("yidx", [128, NT * CH], I32, kind="ExternalInput").ap()
    PT_d = nc.dram_tensor("PT", [128, NPAIR * 128], F16, kind="ExternalInput").ap()
    QTh_d = nc.dram_tensor("QTh", [128, A * 128], F16, kind="ExternalInput").ap()
    QTl_d = nc.dram_tensor("QTl", [128, A * 128], F16, kind="ExternalInput").ap()
    cmat_d = nc.dram_tensor("cmat", [A, D], BF16, kind="ExternalInput").ap()
    armv_d = nc.dram_tensor("armv", [128, NPAIR], F32, kind="ExternalInput").ap()
    iotaA_d = nc.dram_tensor("iotaA", [128, A], F32, kind="ExternalInput").ap()
    iotaCb_d = nc.dram_tensor("iotaCb", [A, 1], BF16, kind="ExternalInput").ap()
    ident_d = nc.dram_tensor("ident", [128, 128], F32, kind="ExternalInput").ap()

    r_out = nc.dram_tensor("r_out", [BC, D], F32, kind="ExternalOutput").ap()
    w_out = nc.dram_tensor("w_out", [BC, D], F32, kind="ExternalOutput").ap()

    RELU = mybir.ActivationFunctionType.Relu
    EQ = mybir.AluOpType.is_equal
    ADD = mybir.AluOpType.add
    SUB = mybir.AluOpType.subtract
    MAX = mybir.AluOpType.max
    MULT = mybir.AluOpType.mult

    with tile.TileContext(nc) as tc, ExitStack() as ctx:
        const = ctx.enter_context(tc.tile_pool(name="const", bufs=1))
        gpool = ctx.enter_context(tc.tile_pool(name="g", bufs=2))
        cpool = ctx.enter_context(tc.tile_pool(name="ctxp", bufs=2))
        zpool = ctx.enter_context(tc.tile_pool(name="zr", bufs=3))
        rpool = ctx.enter_context(tc.tile_pool(name="rr", bufs=3))
        mpool = ctx.enter_context(tc.tile_pool(name="mask", bufs=2))
        spool = ctx.enter_context(tc.tile_pool(name="small", bufs=3))
        opool = ctx.enter_context(tc.tile_pool(name="outs", bufs=2))
        ps_z = ctx.enter_context(tc.tile_pool(name="psz", bufs=3, space="PSUM"))
        ps_p = ctx.enter_context(tc.tile_pool(name="psp", bufs=2, space="PSUM"))
        ps_t = ctx.enter_context(tc.tile_pool(name="pst", bufs=3, space="PSUM"))

        def load_const(name, dram_ap, shape, dtype=F32):
            t = const.tile(shape, dtype, tag=name)
            nc.sync.dma_start(out=t[:], in_=dram_ap)
            return t

        PT = load_const("PT", PT_d, [128, NPAIR * 128], F16)
        QTh = load_const("QTh", QTh_d, [128, A * 128], F16)
        QTl = load_const("QTl", QTl_d, [128, A * 128], F16)
        cmat = load_const("cmat", cmat_d, [A, D], BF16)
        armv = load_const("armv", armv_d, [128, NPAIR])
        iotaCb = load_const("iotaCb", iotaCb_d, [A, 1], BF16)
        iotaA = load_const("iotaA", iotaA_d, [128, A])
        ident = load_const("ident", ident_d, [128, 128])
        xs0 = load_const("xs0", xidx0_d, [128, NT * CH], I32)
        xs1 = load_const("xs1", xidx1_d, [128, NT * CH], I32)
        ys = load_const("ys", yidx_d, [128, NT * CH], I32)

        def gathers(t):
            gx0 = gpool.tile([128, CH, D], F32, tag="gx0")
            gx1 = gpool.tile([128, CH, D], F32, tag="gx1")
            gy = gpool.tile([128, CH, D], F32, tag="gy")
            if batched_gather:
                c0 = t * CH
                nc.gpsimd.indirect_dma_start(
                    out=gx0[:], out_offset=None, in_=cemb,
                    in_offset=bass.IndirectOffsetOnAxis(
                        ap=xs0[:, c0 : c0 + CH], axis=0),
                )
                nc.gpsimd.indirect_dma_start(
                    out=gx1[:], out_offset=None, in_=cemb,
                    in_offset=bass.IndirectOffsetOnAxis(
                        ap=xs1[:, c0 : c0 + CH], axis=0),
                )
                nc.gpsimd.indirect_dma_start(
                    out=gy[:], out_offset=None, in_=wembt,
                    in_offset=bass.IndirectOffsetOnAxis(
                        ap=ys[:, c0 : c0 + CH], axis=0),
                )
            else:
                for j in range(CH):
                    col = t * CH + j
                    nc.gpsimd.indirect_dma_start(
                        out=gx0[:, j, :], out_offset=None, in_=cemb,
                        in_offset=bass.IndirectOffsetOnAxis(
                            ap=xs0[:, col : col + 1], axis=0),
                    )
                    nc.gpsimd.indirect_dma_start(
                        out=gx1[:, j, :], out_offset=None, in_=cemb,
                        in_offset=bass.IndirectOffsetOnAxis(
                            ap=xs1[:, col : col + 1], axis=0),
                    )
                    nc.gpsimd.indirect_dma_start(
                        out=gy[:, j, :], out_offset=None, in_=wembt,
                        in_offset=bass.IndirectOffsetOnAxis(
                            ap=ys[:, col : col + 1], axis=0),
                    )
            w_slice = w_out[t * TILE : (t + 1) * TILE, :].rearrange(
                "(j p) d -> p j d", p=128
            )
            nc.sync.dma_start(out=w_slice, in_=gy[:])
            return gx0, gx1

        def produce(t, gx0, gx1):
            """ctxT, transposed-z rewards, argmin, one-hot S + masks."""
            st = {}
            ctxh = cpool.tile([128, TILE], F16, tag="ctxh")
            ctxl = cpool.tile([128, TILE], F16, tag="ctxl")
            ctp0 = ps_t.tile([64, TILE], F32, tag="misc")
            ctp1 = ps_t.tile([64, TILE], F32, tag="misc")
            for j in range(CH):
                sl = slice(j * 128, (j + 1) * 128)
                nc.tensor.transpose(
                    out=ctp0[:, sl], in_=gx0[:, j, :], identity=ident[:]
                )
                nc.tensor.transpose(
                    out=ctp1[:, sl], in_=gx1[:, j, :], identity=ident[:]
                )
            nc.scalar.copy(out=ctxh[0:64, :], in_=ctp0[:])
            nc.scalar.copy(out=ctxh[64:128, :], in_=ctp1[:])
            nc.vector.tensor_tensor(
                out=ctxl[0:64, :], in0=ctp0[:], in1=ctxh[0:64, :], op=SUB
            )
            nc.vector.tensor_tensor(
                out=ctxl[64:128, :], in0=ctp1[:], in1=ctxh[64:128, :], op=SUB
            )
            st["ctxh"] = ctxh

            S = spool.tile([A, TILE], BF16, tag="S")
            for c in range(CH):
                csl = slice(c * 128, (c + 1) * 128)
                zrs = zpool.tile([128, NG, TILE], F32, tag="zrs")
                for g in range(NG):
                    zps = ps_z.tile([128, TILE], F32, tag="z")
                    gsl = slice(g * TILE, (g + 1) * TILE)
                    nc.tensor.matmul(
                        out=zps[:], lhsT=ctxh[:, csl], rhs=QTh[:, gsl],
                        start=True, stop=False,
                    )
                    nc.tensor.matmul(
                        out=zps[:], lhsT=ctxh[:, csl], rhs=QTl[:, gsl],
                        start=False, stop=False,
                    )
                    nc.tensor.matmul(
                        out=zps[:], lhsT=ctxl[:, csl], rhs=QTh[:, gsl],
                        start=False, stop=True,
                    )
                    if g < ACT_RELU:
                        nc.scalar.activation(
                            out=zrs[:, g, :], in_=zps[:], func=RELU,
                            bias=0.0, scale=1.0,
                        )
                    else:
                        nc.vector.tensor_scalar(
                            out=zrs[:, g, :], in0=zps[:],
                            scalar1=0.0, scalar2=None, op0=MAX,
                        )
                # rewards = sum(relu pos block) - sum(relu neg block);
                # nr = -rewards so the argmin becomes an argmax.
                nr_c = rpool.tile([128, A], F32, tag="nr")
                zv = zrs[:].rearrange("q g (a h) -> q (g a) h", a=4, h=128)
                if p == 0:
                    nc.vector.tensor_reduce(
                        out=nr_c[:], in_=zv, axis=mybir.AxisListType.X, op=ADD
                    )
                elif p == 128:
                    rpos = rpool.tile([128, A], F32, tag="rpos")
                    nc.vector.tensor_reduce(
                        out=rpos[:], in_=zv, axis=mybir.AxisListType.X, op=ADD
                    )
                    nc.vector.tensor_scalar(
                        out=nr_c[:], in0=rpos[:], scalar1=-1.0, scalar2=None,
                        op0=MULT,
                    )
                else:
                    rpos = rpool.tile([128, A], F32, tag="rpos")
                    rneg = rpool.tile([128, A], F32, tag="rneg")
                    nc.vector.tensor_reduce(
                        out=rpos[:], in_=zv[:, :, 0:p],
                        axis=mybir.AxisListType.X, op=ADD,
                    )
                    nc.vector.tensor_reduce(
                        out=rneg[:], in_=zv[:, :, p:128],
                        axis=mybir.AxisListType.X, op=ADD,
                    )
                    nc.vector.tensor_tensor(
                        out=nr_c[:], in0=rneg[:], in1=rpos[:], op=SUB
                    )
                mx8 = spool.tile([128, 8], F32, tag="mx8")
                ix8 = spool.tile([128, 8], U32, tag="ix8")
                nc.vector.max(out=mx8[:], in_=nr_c[:])
                nc.vector.max_index(out=ix8[:], in_max=mx8[:], in_values=nr_c[:])
                ixf = spool.tile([128, 1], F32, tag="ixf")
                nc.vector.tensor_copy(out=ixf[:], in_=ix8[:, 0:1])
                oh = spool.tile([128, A], F32, tag="oh")
                nc.vector.tensor_scalar(
                    out=oh[:], in0=iotaA[:], scalar1=ixf[:], scalar2=None, op0=EQ
                )
                Sps = ps_t.tile([A, 128], F32, tag="misc")
                nc.tensor.transpose(out=Sps[:], in_=oh[:], identity=ident[:])
                nc.scalar.copy(out=S[:, csl], in_=Sps[:])
            st["S"] = S

            ixTps = ps_t.tile([1, TILE], F32, tag="misc")
            nc.tensor.matmul(
                out=ixTps[:], lhsT=iotaCb[:], rhs=S[:], start=True, stop=True
            )
            ixT = spool.tile([1, TILE], BF16, tag="ixT")
            nc.scalar.copy(out=ixT[:], in_=ixTps[:])
            ixB = mpool.tile([128, TILE], BF16, tag="ixB")
            nc.gpsimd.partition_broadcast(ixB[:], ixT[:], channels=128)
            masks = mpool.tile([128, NPAIR, TILE], U16, tag="masks")
            st["masks"] = masks
            for j in range(NPAIR):
                nc.vector.tensor_scalar(
                    out=masks[:, j, :], in0=ixB[:],
                    scalar1=armv[:, j : j + 1], scalar2=None, op0=EQ,
                )
            return st

        def consume(t, st):
            """preds (straight from PSUM) + predicated select + store.
            """
            S = st["S"]
            masks = st["masks"]
            ctxh = st["ctxh"]
            cselp = ps_t.tile([D, TILE], F32, tag="misc")
            nc.tensor.matmul(
                out=cselp[:], lhsT=cmat[:], rhs=S[:], start=True, stop=True
            )
            csel = opool.tile([D, TILE], F32, tag="csel")
            nc.scalar.copy(out=csel[:], in_=cselp[:])

            rsel2 = opool.tile([128, TILE], F32, tag="rsel2")
            nc.vector.memset(rsel2[:], 0.0)
            for j in range(NPAIR):
                pps = ps_p.tile([128, TILE], F32, tag="pp")
                nc.tensor.matmul(
                    out=pps[:], lhsT=PT[:, j * 128 : (j + 1) * 128],
                    rhs=ctxh[:], start=True, stop=True,
                )
                nc.vector.copy_predicated(
                    out=rsel2[:], mask=masks[:, j, :], data=pps[:]
                )
            rte = opool.tile([D, TILE], F32, tag="rte")
            nc.vector.tensor_tensor(
                out=rte[:], in0=rsel2[0:64, :], in1=csel[:], op=ADD
            )
            for c in range(CH):
                sl = slice(c * 128, (c + 1) * 128)
                tpe = ps_t.tile([128, D], F32, tag="misc")
                nc.tensor.matmul(
                    out=tpe[:], lhsT=rte[:, sl], rhs=ident[0:64, 0:64],
                    is_transpose=True, start=True, stop=True,
                )
                tpo = ps_t.tile([128, D], F32, tag="misc")
                nc.tensor.matmul(
                    out=tpo[:], lhsT=rsel2[64:128, sl],
                    rhs=ident[64:128, 64:128], start=True, stop=True,
                )
                tse = opool.tile([128, D], F32, tag="tse")
                nc.scalar.copy(out=tse[:], in_=tpe[:])
                rts = opool.tile([128, D], F32, tag="rts")
                nc.vector.tensor_tensor(
                    out=rts[:], in0=tse[:], in1=tpo[:], op=ADD
                )
                base = t * TILE + c * 128
                nc.sync.dma_start(out=r_out[base : base + 128, :], in_=rts[:])

        prev = None
        for t in range(NT):
            gx = gathers(t)
            if prev is not None:
                consume(t - 1, prev)
            prev = produce(t, *gx)
        consume(NT - 1, prev)

    nc.compile()
    return nc


def _build_v1(loop_reps=1, upto=7, preds_f32r=True, preds_copy='split'):
    """General-bias fallback (original program)."""
    nc = bacc.Bacc(
        "TRN2", target_bir_lowering=False, debug=False, num_devices=NCORES
    )

    cemb = nc.dram_tensor("cemb", [V, D], F32, kind="ExternalInput").ap()
    wembt = nc.dram_tensor("wembt", [V, D], F32, kind="ExternalInput").ap()
    xidx0_d = nc.dram_tensor("xidx0", [128, NT * CH], I32, kind="ExternalInput").ap()
    xidx1_d = nc.dram_tensor("xidx1", [128, NT * CH], I32, kind="ExternalInput").ap()
    yidx_d = nc.dram_tensor("yidx", [128, NT * CH], I32, kind="ExternalInput").ap()
    PT_d = nc.dram_tensor("PT", [128, NPAIR * 128], F32R, kind="ExternalInput").ap()
    QT_d = nc.dram_tensor("QT", [128, A * 128], F32, kind="ExternalInput").ap()
    dmat_d = nc.dram_tensor("dmat", [128, A], F32, kind="ExternalInput").ap()
    negw_d = nc.dram_tensor("negw", [128, 1], F32, kind="ExternalInput").ap()
    cmat_d = nc.dram_tensor("cmat", [A, D], F32, kind="ExternalInput").ap()
    armv_d = nc.dram_tensor("armv", [128, NPAIR], F32, kind="ExternalInput").ap()
    iotaA_d = nc.dram_tensor("iotaA", [128, A], F32, kind="ExternalInput").ap()
    iotaC_d = nc.dram_tensor("iotaC", [A, 1], F32, kind="ExternalInput").ap()
    ident_d = nc.dram_tensor("ident", [128, 128], F32, kind="ExternalInput").ap()

    r_out = nc.dram_tensor("r_out", [BC, D], F32, kind="ExternalOutput").ap()
    w_out = nc.dram_tensor("w_out", [BC, D], F32, kind="ExternalOutput").ap()

    RELU = mybir.ActivationFunctionType.Relu
    EQ = mybir.AluOpType.is_equal
    ADD = mybir.AluOpType.add

    with tile.TileContext(nc) as tc, ExitStack() as ctx:
        const = ctx.enter_context(tc.tile_pool(name="const", bufs=1))
        gpool = ctx.enter_context(tc.tile_pool(name="g", bufs=3))
        cpool = ctx.enter_context(tc.tile_pool(name="ctxp", bufs=2))
        zpool = ctx.enter_context(tc.tile_pool(name="zr", bufs=4))
        mpool = ctx.enter_context(tc.tile_pool(name="mask", bufs=2))
        spool = ctx.enter_context(tc.tile_pool(name="small", bufs=2))
        opool = ctx.enter_context(tc.tile_pool(name="outs", bufs=2))
        ppool = ctx.enter_context(tc.tile_pool(name="predsp", bufs=2))
        ps_z = ctx.enter_context(tc.tile_pool(name="psz", bufs=2, space="PSUM"))
        ps_p = ctx.enter_context(tc.tile_pool(name="psp", bufs=2, space="PSUM"))
        ps_nr = ctx.enter_context(tc.tile_pool(name="psnr", bufs=2, space="PSUM"))
        ps_t = ctx.enter_context(tc.tile_pool(name="pst", bufs=2, space="PSUM"))

        def load_const(name, dram_ap, shape, dtype=F32):
            t = const.tile(shape, dtype, tag=name)
            nc.sync.dma_start(out=t[:], in_=dram_ap)
            return t

        PT = load_const("PT", PT_d, [128, NPAIR * 128], F32R)
        QT = load_const("QT", QT_d, [128, A * 128])
        dmat = load_const("dmat", dmat_d, [128, A])
        negw = load_const("negw", negw_d, [128, 1])
        cmat = load_const("cmat", cmat_d, [A, D])
        armv = load_const("armv", armv_d, [128, NPAIR])
        iotaA = load_const("iotaA", iotaA_d, [128, A])
        iotaC = load_const("iotaC", iotaC_d, [A, 1])
        ident = load_const("ident", ident_d, [128, 128])
        xs0 = load_const("xs0", xidx0_d, [128, NT * CH], I32)
        xs1 = load_const("xs1", xidx1_d, [128, NT * CH], I32)
        ys = load_const("ys", yidx_d, [128, NT * CH], I32)

        def produce(t):
            st = {}
            gx0 = gpool.tile([128, CH, D], F32, tag="gx0")
            gx1 = gpool.tile([128, CH, D], F32, tag="gx1")
            gy = gpool.tile([128, CH, D], F32, tag="gy")
            for j in range(CH):
                col = t * CH + j
                nc.gpsimd.indirect_dma_start(
                    out=gx0[:, j, :], out_offset=None, in_=cemb,
                    in_offset=bass.IndirectOffsetOnAxis(
                        ap=xs0[:, col : col + 1], axis=0),
                )
                nc.gpsimd.indirect_dma_start(
                    out=gx1[:, j, :], out_offset=None, in_=cemb,
                    in_offset=bass.IndirectOffsetOnAxis(
                        ap=xs1[:, col : col + 1], axis=0),
                )
                nc.gpsimd.indirect_dma_start(
                    out=gy[:, j, :], out_offset=None, in_=wembt,
                    in_offset=bass.IndirectOffsetOnAxis(
                        ap=ys[:, col : col + 1], axis=0),
                )
            w_slice = w_out[t * TILE : (t + 1) * TILE, :].rearrange(
                "(j p) d -> p j d", p=128
            )
            nc.sync.dma_start(out=w_slice, in_=gy[:])

            ctxT = cpool.tile([128, TILE], F32, tag="ctxT")
            ctp0 = ps_t.tile([64, TILE], F32, tag="misc")
            ctp1 = ps_t.tile([64, TILE], F32, tag="misc")
            for j in range(CH):
                sl = slice(j * 128, (j + 1) * 128)
                nc.tensor.transpose(
                    out=ctp0[:, sl], in_=gx0[:, j, :], identity=ident[:]
                )
                nc.tensor.transpose(
                    out=ctp1[:, sl], in_=gx1[:, j, :], identity=ident[:]
                )
            nc.vector.tensor_copy(out=ctxT[0:64, :], in_=ctp0[:])
            nc.vector.tensor_copy(out=ctxT[64:128, :], in_=ctp1[:])
            st["ctxT"] = ctxT
            ctxTr = cpool.tile([128, TILE], F32R, tag="ctxTr")
            nc.vector.tensor_copy(out=ctxTr[:], in_=ctxT[:])
            st["ctxTr"] = ctxTr

            nr = ps_nr.tile([128, CH, A], F32, tag="nr")
            st["nr"] = nr
            zr_tiles = {}
            for a in range(A):
                zps = ps_z.tile([128, TILE], F32, tag="z")
                nc.tensor.matmul(
                    out=zps[:], lhsT=QT[:, a * 128 : (a + 1) * 128],
                    rhs=ctxT[:], start=True, stop=True,
                )
                zr = zpool.tile([128, TILE], F32, tag="zr")
                nc.scalar.activation(
                    out=zr[:], in_=zps[:], func=RELU,
                    bias=dmat[:, a : a + 1], scale=1.0,
                )
                zr_tiles[a] = zr
                if a > 0:
                    zp = zr_tiles.pop(a - 1)
                    for c in range(CH):
                        nc.tensor.matmul(
                            out=nr[:, c, a - 1 : a],
                            lhsT=zp[:, c * 128 : (c + 1) * 128],
                            rhs=negw[:], start=True, stop=True,
                        )
            zp = zr_tiles.pop(A - 1)
            for c in range(CH):
                nc.tensor.matmul(
                    out=nr[:, c, A - 1 : A],
                    lhsT=zp[:, c * 128 : (c + 1) * 128],
                    rhs=negw[:], start=True, stop=True,
                )

            preds_sb = ppool.tile([128, NPAIR, TILE], F32, tag="preds")
            st["preds"] = preds_sb
            for j in range(NPAIR):
                pps = ps_p.tile([128, TILE], F32, tag="pp")
                nc.tensor.matmul(
                    out=pps[:], lhsT=PT[:, j * 128 : (j + 1) * 128],
                    rhs=st["ctxTr"][:], start=True, stop=True,
                )
                if j % 2 == 0:
                    nc.scalar.copy(out=preds_sb[:, j, :], in_=pps[:])
                else:
                    nc.vector.tensor_copy(out=preds_sb[:, j, :], in_=pps[:])
            return st

        def route(t, st):
            nr = st["nr"]
            S = spool.tile([A, TILE], F32, tag="S")
            for c in range(CH):
                nrs = spool.tile([128, A], F32, tag="nrs")
                nc.vector.tensor_copy(out=nrs[:], in_=nr[:, c, :])
                mx8 = spool.tile([128, 8], F32, tag="mx8")
                ix8 = spool.tile([128, 8], U32, tag="ix8")
                nc.vector.max(out=mx8[:], in_=nrs[:])
                nc.vector.max_index(out=ix8[:], in_max=mx8[:], in_values=nrs[:])
                ixf = spool.tile([128, 1], F32, tag="ixf")
                nc.vector.tensor_copy(out=ixf[:], in_=ix8[:, 0:1])
                oh = spool.tile([128, A], F32, tag="oh")
                nc.vector.tensor_scalar(
                    out=oh[:], in0=iotaA[:], scalar1=ixf[:], scalar2=None, op0=EQ
                )
                Sps = ps_t.tile([A, 128], F32, tag="misc")
                nc.tensor.transpose(out=Sps[:], in_=oh[:], identity=ident[:])
                nc.scalar.copy(out=S[:, c * 128 : (c + 1) * 128], in_=Sps[:])

            ixTps = ps_t.tile([1, TILE], F32, tag="misc")
            nc.tensor.matmul(
                out=ixTps[:], lhsT=iotaC[:], rhs=S[:], start=True, stop=True
            )
            ixT = spool.tile([1, TILE], F32, tag="ixT")
            nc.vector.tensor_copy(out=ixT[:], in_=ixTps[:])
            ixB = mpool.tile([128, TILE], F32, tag="ixB")
            nc.gpsimd.partition_broadcast(ixB[:], ixT[:], channels=128)
            masks = mpool.tile([128, NPAIR, TILE], U16, tag="masks")
            st["masks"] = masks
            for j in range(NPAIR):
                nc.vector.tensor_scalar(
                    out=masks[:, j, :], in0=ixB[:],
                    scalar1=armv[:, j : j + 1], scalar2=None, op0=EQ,
                )
            cselp = ps_t.tile([D, TILE], F32, tag="misc")
            nc.tensor.matmul(
                out=cselp[:], lhsT=cmat[:], rhs=S[:], start=True, stop=True
            )
            csel = opool.tile([D, TILE], F32, tag="csel")
            nc.scalar.copy(out=csel[:], in_=cselp[:])
            st["csel"] = csel

        def finish(t, st):
            masks = st["masks"]
            preds_sb = st["preds"]
            csel = st["csel"]
            rsel2 = opool.tile([128, TILE], F32, tag="rsel2")
            nc.vector.memset(rsel2[:], 0.0)
            for j in range(NPAIR):
                nc.vector.copy_predicated(
                    out=rsel2[:], mask=masks[:, j, :], data=preds_sb[:, j, :]
                )
            rte = opool.tile([D, TILE], F32, tag="rte")
            nc.vector.tensor_tensor(
                out=rte[:], in0=rsel2[0:64, :], in1=csel[:], op=ADD
            )
            for c in range(CH):
                sl = slice(c * 128, (c + 1) * 128)
                tpe = ps_t.tile([128, D], F32, tag="misc")
                nc.tensor.matmul(
                    out=tpe[:], lhsT=rte[:, sl], rhs=ident[0:64, 0:64],
                    is_transpose=True, start=True, stop=True,
                )
                tpo = ps_t.tile([128, D], F32, tag="misc")
                nc.tensor.matmul(
                    out=tpo[:], lhsT=rsel2[64:128, sl],
                    rhs=ident[64:128, 64:128], start=True, stop=True,
                )
                tse = opool.tile([128, D], F32, tag="tse")
                nc.scalar.copy(out=tse[:], in_=tpe[:])
                rts = opool.tile([128, D], F32, tag="rts")
                nc.vector.tensor_tensor(
                    out=rts[:], in0=tse[:], in1=tpo[:], op=ADD
                )
                base = t * TILE + c * 128
                nc.sync.dma_start(out=r_out[base : base + 128, :], in_=rts[:])

        prev = None
        for t in range(NT):
            if prev is not None:
                route(t - 1, prev)
            st = produce(t)
            if prev is not None:
                finish(t - 1, prev)
            prev = st
        route(NT - 1, prev)
        finish(NT - 1, prev)

    nc.compile()
    return nc


def _host_prep(x, y, cemb_table, wemb_table, W1, b1, W2, b2, Wr1, br1, Wr2, br2):
    """Collapse the per-arm networks (fp64 for accuracy, cast to fp32)."""
    W1_ = W1.astype(np.float64)
    W2_ = W2.astype(np.float64)
    b1_ = b1.astype(np.float64)
    b2_ = b2.astype(np.float64)
    Wc = Wr1[:, :D2].astype(np.float64)  # [H, 2D]
    Wp = Wr1[:, D2:].astype(np.float64)  # [H, D]
    br1_ = br1.astype(np.float64)
    w = Wr2.astype(np.float64)  # [H]

    P = np.einsum("adh,ahi->adi", W2_, W1_)  # [A, D, 2D]
    c = np.einsum("adh,ah->ad", W2_, b1_) + b2_  # [A, D]
    Q = np.einsum("hd,adi->ahi", Wp, P) + Wc[None, :, :]  # [A, H, 2D]
    dv = np.einsum("hd,ad->ah", Wp, c) + br1_[None, :]  # [A, H]

    PT = np.concatenate(
        [
            np.concatenate([P[2 * j].T, P[2 * j + 1].T], axis=1)
            for j in range(NPAIR)
        ],
        axis=1,
    ).astype(np.float32)  # [2D, NPAIR*128]
    cmat = c.astype(np.float32)  # [A, D]

    iotaA = np.tile(np.arange(A, dtype=np.float32)[None, :], (128, 1))
    iotaC = np.arange(A, dtype=np.float32)[:, None]
    ident = np.eye(128, dtype=np.float32)

    x32 = np.ascontiguousarray(np.asarray(x).astype(np.int32))
    y32 = np.ascontiguousarray(np.asarray(y).astype(np.int32))

    def idx_layout(v):  # [BC] -> [128, NT*CH] with col t*CH+j, row p
        return np.ascontiguousarray(
            v.reshape(NT, CH, 128).transpose(2, 0, 1).reshape(128, NT * CH)
        )

    use_v2 = bool(np.allclose(dv, 0.0, atol=1e-12))

    shared = dict(
        cemb=np.ascontiguousarray(cemb_table.astype(np.float32)),
        wembt=np.ascontiguousarray(wemb_table.astype(np.float32)),
        PT=np.ascontiguousarray(PT),
        cmat=np.ascontiguousarray(cmat),
        iotaA=np.ascontiguousarray(iotaA),
        iotaC=iotaC,
        ident=ident,
    )
    if use_v2:
        # sort H dims: w>0 first; fold |w| into the Q rows
        perm = np.argsort(w <= 0, kind="stable")
        p = int((w > 0).sum())
        Qs = (np.abs(w)[None, :, None] * Q)[:, perm, :]  # [A, H, 2D] fp64
        # per-arm columns [pos rows(p) | neg rows(H-p)] so the sign-split
        # reduces see a uniform arm stride of 128
        QTs = np.concatenate([Qs[a].T for a in range(A)], axis=1)  # [2D, A*128]
        QTh = QTs.astype(np.float16)
        QTl = (QTs - QTh.astype(np.float64)).astype(np.float16)
        import ml_dtypes

        armv = np.empty((128, NPAIR), np.float32)
        for j in range(NPAIR):
            armv[:64, j] = 2 * j
            armv[64:, j] = 2 * j + 1
        shared["QTh"] = np.ascontiguousarray(QTh)
        shared["QTl"] = np.ascontiguousarray(QTl)
        shared["PT"] = np.ascontiguousarray(
            shared["PT"].astype(np.float16)
        )
        shared["armv"] = armv
        shared["iotaCb"] = np.ascontiguousarray(
            iotaC.astype(ml_dtypes.bfloat16)
        )
        shared["cmat"] = np.ascontiguousarray(
            shared["cmat"].astype(ml_dtypes.bfloat16)
        )
        del shared["iotaC"]
        variant = ("v2", p)
    else:
        QT = np.concatenate([Q[a].T for a in range(A)], axis=1).astype(
            np.float32
        )  # [2D, A*128]
        dmat = dv.T.astype(np.float32)  # [H, A]
        negw = (-w)[:, None].astype(np.float32)  # [H, 1]
        armv = np.empty((128, NPAIR), np.float32)
        for j in range(NPAIR):
            armv[:64, j] = 2 * j
            armv[64:, j] = 2 * j + 1
        shared["QT"] = np.ascontiguousarray(QT)
        shared["dmat"] = np.ascontiguousarray(dmat)
        shared["negw"] = np.ascontiguousarray(negw)
        shared["armv"] = armv
        variant = ("v1",)

    in_maps = []
    for k in range(NCORES):
        lo, hi = k * BC, (k + 1) * BC
        m = dict(shared)
        m["xidx0"] = idx_layout(x32[lo:hi, 0])
        m["xidx1"] = idx_layout(x32[lo:hi, 1])
        m["yidx"] = idx_layout(y32[lo:hi])
        in_maps.append(m)
    return in_maps, variant


def _get_nc(variant):
    if variant not in _CACHE:
        if variant[0] == "v2":
            _CACHE[variant] = _build_v2(variant[1])
        else:
            _CACHE[variant] = _build_v1()
    return _CACHE[variant]


def run(inputs, trace=False, **kw):
    """Build + execute; returns (outputs_tuple, BassKernelResults)."""
    in_maps, variant = _host_prep(**{k: np.asarray(v) for k, v in inputs.items()})
    nc = _get_nc(variant)
    res = run_bass_kernel_spmd(nc, in_maps, list(range(NCORES)), trace=trace, **kw)
    r_full = np.concatenate([res.results[k]["r_out"] for k in range(NCORES)], axis=0)
    w_full = np.concatenate([res.results[k]["w_out"] for k in range(NCORES)], axis=0)
    return (r_full, w_full), res


def kernel(**inputs):
    out, _ = run(inputs)
    return out
